# revision 3
# baseline (speedup 1.0000x reference)
"""Trainium2 Bass kernel for nn_MinibatchDiscriminator (N=512, INSIZE=512, K=64, D=16).

Symmetric pair-coverage scheme: each core owns a 64-row block and a 320-col
window of j (own block + next 3 blocks + half of the opposite block, chosen
so every unordered (i, j) pair is computed exactly once).  Per (i, chunk):
one elementwise op — DVE tensor_scalar min(f_j, f_i) at 4x mode, or ACT
Abs(|f_j - f_i|) for a load-balancing share.  PE d-reduction (weight +1 for
min rows, -0.5 for abs rows) plus a -B/2 correction from an fp32 B-table
yields bank = M - B/2 (min rows) / -norm/2 (abs rows); ACT exp(2*bank + bias)
with bias = -A (from the B-table, zero on abs rows) accumulates the own-row
sums over j, and a PE partition-sum of the exp tile yields the cross-row
(j-side) contributions.  The diagonal contributes
exactly exp(0)=1 (the scalar column is an exact fp32 copy of the fp16 feat
values, and the B-table stays fp32) and is subtracted on the host.  Host
assembles o_b from per-core partials.
"""
import sys

import numpy as np

sys.path.insert(0, "/opt/trn_rl_repo")

import concourse.bass as bass
import concourse.tile as tile
from concourse import bacc, mybir
from concourse.bass_utils import run_bass_kernel_spmd

AF = mybir.ActivationFunctionType
OP = mybir.AluOpType
FP32 = mybir.dt.float32
FP16 = mybir.dt.float16
BF16 = mybir.dt.bfloat16

N, INSIZE, K, D = 512, 512, 64, 16
KD = K * D
NCORES = 8
NL = N // NCORES          # 64 rows per core
P = 128
CH = KD // P              # 8 chunks of (8k x 16d)
NT = INSIZE // P          # 4 contraction tiles
WIN = 320                 # feat window cols
WOP = 288                 # per-op j width
NP = 16                   # i-groups of 4 (p), i = 4p + C

TRACE = False
_cache = {}


BIG = 200.0


def _act_unit(p, C, c):
    """Static DVE/ACT split for the |diff| units (tune for balance)."""
    return c >= 6


def _build():
    nc = bacc.Bacc("TRN2", target_bir_lowering=False)
    xtw_h = nc.dram_tensor("xtw", [NT, P, WIN], FP16, kind="ExternalInput").ap()
    wt_h = nc.dram_tensor("wt", [CH, P, NT, P], FP16, kind="ExternalInput").ap()
    w64min_h = nc.dram_tensor("w64min", [P, CH * 32], FP16, kind="ExternalInput").ap()
    w64abs_h = nc.dram_tensor("w64abs", [P, CH * 32], FP16, kind="ExternalInput").ap()
    bhalf_h = nc.dram_tensor("bhalf", [P, 3, 32], FP16, kind="ExternalInput").ap()
    wneg_h = nc.dram_tensor("wneg", [P, CH * 32], FP16, kind="ExternalInput").ap()
    selw_h = nc.dram_tensor("selw", [P, 2, 64], BF16, kind="ExternalInput").ap()
    zb_h = nc.dram_tensor("zb", [1, 64 + WIN], BF16, kind="ExternalInput").ap()
    bigw_h = nc.dram_tensor("bigw", [P, 4 * 32], FP16, kind="ExternalInput").ap()
    ebuf_h = nc.dram_tensor("ebuf", [P, 608], FP16, kind="ExternalInput").ap()
    xs_h = nc.dram_tensor("xs", [NL, INSIZE], FP32, kind="ExternalInput").ap()
    outx_h = nc.dram_tensor("outx", [NL, INSIZE], FP32, kind="ExternalOutput").ap()
    oraw_h = nc.dram_tensor("oraw", [P, 2 * NP], FP32, kind="ExternalOutput").ap()
    objo_h = nc.dram_tensor("objo", [64, WIN], FP32, kind="ExternalOutput").ap()

    with tile.TileContext(nc) as tc:
        with (
            tc.tile_pool(name="cst", bufs=1) as cst,
            tc.tile_pool(name="ad", bufs=128) as adp,
            tc.tile_pool(name="scr", bufs=8) as scp,
            tc.tile_pool(name="fps", bufs=2, space="PSUM") as fps,
            tc.tile_pool(name="nps", bufs=2, space="PSUM") as nps,
            tc.tile_pool(name="ops", bufs=1, space="PSUM") as ops,
        ):
            # ---------------- input DMAs (split across queues) ----------------
            xtw_sb = []
            for t in range(NT):
                s = cst.tile([P, WIN], FP16, tag=f"xtw{t}")
                (nc.sync if t % 2 == 0 else nc.gpsimd).dma_start(out=s, in_=xtw_h[t])
                xtw_sb.append(s)
            wt_sb = []
            for c in range(CH):
                s = cst.tile([P, NT, P], FP16, tag=f"wt{c}")
                (nc.sync if c % 2 == 0 else nc.gpsimd).dma_start(out=s, in_=wt_h[c])
                wt_sb.append(s)
            w64min_sb = cst.tile([P, CH * 32], FP16, tag="w64min")
            nc.sync.dma_start(out=w64min_sb, in_=w64min_h)
            w64abs_sb = cst.tile([P, CH * 32], FP16, tag="w64abs")
            nc.gpsimd.dma_start(out=w64abs_sb, in_=w64abs_h)
            bhalf_sb = cst.tile([P, 3, 32], FP16, tag="bhalf")
            nc.sync.dma_start(out=bhalf_sb, in_=bhalf_h)
            wneg_sb = cst.tile([P, CH * 32], FP16, tag="wneg")
            nc.gpsimd.dma_start(out=wneg_sb, in_=wneg_h)
            selw_sb = cst.tile([P, 2, 64], BF16, tag="selw")
            nc.sync.dma_start(out=selw_sb, in_=selw_h)
            zb_sb = cst.tile([1, 64 + WIN], BF16, tag="zb")
            nc.gpsimd.dma_start(out=zb_sb, in_=zb_h)
            bigw_sb = cst.tile([P, 4 * 32], FP16, tag="bigw")
            nc.sync.dma_start(out=bigw_sb, in_=bigw_h)
            ebuf_sb = cst.tile([P, 608], FP16, tag="ebuf")
            nc.gpsimd.dma_start(out=ebuf_sb, in_=ebuf_h)
            nc.gpsimd.dma_start(out=outx_h, in_=xs_h)

            o_raw = cst.tile([P, 2 * NP], FP32, tag="o_raw")

            # ---------------- feat window per chunk ----------------
            featH, fCol = [None] * CH, [None] * CH
            for c in range(CH):
                psf = fps.tile([P, WIN], FP32, tag="psf", name=f"psf{c}")
                for t in range(NT):
                    nc.tensor.matmul(
                        psf, wt_sb[c][:, t, :], xtw_sb[t],
                        start=(t == 0), stop=(t == NT - 1),
                    )
                fh = cst.tile([P, WIN], FP16, tag=f"fh{c}")
                nc.scalar.copy(fh, psf)
                featH[c] = fh
                # fp32 scalar columns == exact fp32 copies of the fp16 values
                fc = cst.tile([P, NL], FP32, tag=f"fc{c}")
                nc.vector.tensor_copy(fc, fh[:, 32 : 32 + NL])
                fCol[c] = fc

            # ---------------- B table (fp32): B[k] = sum_d f[k, d] ----------------
            btp = fps.tile([P, WIN], FP32, tag="psf", name="btp")
            for h in range(2):
                for c in range(CH):
                    nc.tensor.matmul(
                        btp[64 * h : 64 * h + 32, :],
                        wneg_sb[64 * h : 64 * h + 64, 32 * c : 32 * c + 32],
                        featH[c][64 * h : 64 * h + 64, :],
                        start=(c == 0), stop=(c == CH - 1),
                        tile_position=(64 * h, 64 * h),
                    )
            btile = cst.tile([P, WIN], FP16, tag="btile")
            nc.vector.tensor_copy(btile, btp)
            negA = cst.tile([P, 2 * NP], FP32, tag="negA")
            nc.gpsimd.memset(negA, 0.0)
            for C in range(4):
                rmin = 4 * sum(1 for c in range(CH) if not _act_unit(0, C, c))
                for h in range(2):
                    nc.gpsimd.dma_start(
                        out=negA[32 * C : 32 * C + rmin, h : h + 2 * NP - 1 : 2],
                        in_=btile[64 * h : 64 * h + rmin, 32 + C : 32 + C + 61 : 4],
                    )

            # obJ accumulator bracket: zero-weight matmul sets has_written
            obj_ps = ops.tile([64, WIN], FP32, tag="obj")
            nc.tensor.matmul(
                obj_ps, zb_sb[:, 0:64], zb_sb[:, 64 : 64 + WIN],
                start=True, stop=False,
            )

            # ---------------- p-loop ----------------
            scr_q = {}
            banks_q = {}

            def emit_exp(p):
                for h in range(2):
                    scr = scp.tile([P, WOP], BF16, tag="scr", name="scr")
                    nc.scalar.activation(
                        scr, banks_q.pop((p, h)), AF.Exp,
                        bias=negA[:, 2 * p + h : 2 * p + h + 1], scale=2.0,
                        accum_out=o_raw[:, 2 * p + h : 2 * p + h + 1],
                    )
                    scr_q[(p, h)] = scr

            def emit_b(p):
                for h in range(2):
                    scr = scr_q.pop((p, h))
                    if p < 8:
                        nc.tensor.matmul(
                            obj_ps[:, 96:WIN], selw_sb[:, h, :], scr[:, 64:WOP],
                            start=False, stop=False,
                        )
                    else:
                        nc.tensor.matmul(
                            obj_ps[:, 0:32], selw_sb[:, h, :], scr[:, 0:32],
                            start=False, stop=False,
                        )
                        nc.tensor.matmul(
                            obj_ps[:, 96:WOP], selw_sb[:, h, :], scr[:, 96:WOP],
                            start=False, stop=False,
                        )

            for p in range(NP):
                opoff = 32 if p < 8 else 0
                ads = {}
                for c in range(CH):
                    for C in range(4):
                        i = 4 * p + C
                        src = featH[c][:, opoff : opoff + WOP]
                        col = fCol[c][:, i : i + 1]
                        ad = adp.tile([P, WOP], FP16, tag="ad", name="ad")
                        if _act_unit(p, C, c):
                            nc.scalar.activation(ad, src, AF.Abs, bias=col, scale=-1.0)
                        else:
                            nc.vector.tensor_scalar(
                                ad, src, col, None, op0=OP.min
                            )
                        ads[(C, c)] = ad
                banks = []
                for h in range(2):
                    bk = nps.tile([P, WOP], FP32, tag=f"nb{h}", name=f"nb{h}")
                    banks.append(bk)
                    banks_q[(p, h)] = bk
                for c in range(CH):
                    for C in range(4):
                        wsel = w64abs_sb if _act_unit(p, C, c) else w64min_sb
                        for h in range(2):
                            nc.tensor.matmul(
                                banks[h][32 * C : 32 * C + 32, :],
                                wsel[64 * h : 64 * h + 64, 32 * c : 32 * c + 32],
                                ads[(C, c)][64 * h : 64 * h + 64, :],
                                start=(c == 0), stop=False,
                                tile_position=(64 * h, 32 * C),
                            )
                # -B/2 correction on the min rows (bhalf diag is +0.5, btile=-B)
                for C in range(4):
                    v = 1 if _act_unit(p, C, 6) else 0
                    for h in range(2):
                        nc.tensor.matmul(
                            banks[h][32 * C : 32 * C + 32, :],
                            bhalf_sb[64 * h : 64 * h + 64, v, :],
                            btile[64 * h : 64 * h + 64, opoff : opoff + WOP],
                            start=False, stop=False,
                            tile_position=(64 * h, 32 * C),
                        )
                # eraser: -BIG at the window diagonal col
                pos0 = (4 * p) if p < 8 else (32 + 4 * p)
                for C in range(4):
                    for h in range(2):
                        off = 320 - (pos0 + C)
                        nc.tensor.matmul(
                            banks[h][32 * C : 32 * C + 32, :],
                            bigw_sb[64 * h : 64 * h + 64, 32 * C : 32 * C + 32],
                            ebuf_sb[64 * h : 64 * h + 64, off : off + WOP],
                            start=False, stop=True,
                            tile_position=(64 * h, 32 * C),
                        )
                # exp deferred 1 iter (avoids ACT head-of-line stalls);
                # (b) matmuls deferred 2 iters (avoids PE stalls)
                if p >= 1:
                    emit_exp(p - 1)
                if p >= 2:
                    emit_b(p - 2)
            emit_exp(NP - 1)
            emit_b(NP - 2)
            emit_b(NP - 1)

            # bracket stop
            nc.tensor.matmul(
                obj_ps, zb_sb[:, 0:64], zb_sb[:, 64 : 64 + WIN],
                start=False, stop=True,
            )

            # ---------------- outputs ----------------
            obj_sb = cst.tile([64, WIN], FP32, tag="objsb")
            nc.vector.tensor_copy(obj_sb, obj_ps)
            nc.sync.dma_start(out=objo_h, in_=obj_sb)
            nc.sync.dma_start(out=oraw_h, in_=o_raw)

    nc.finalize()
    return nc


def _consts():
    # w64min/w64abs: contract row 16m+d of half h -> out col 4c+m
    w64min = np.zeros((2, 64, CH, 32), np.float16)
    w64abs = np.zeros((2, 64, CH, 32), np.float16)
    for a in range(2):
        for c in range(CH):
            for m in range(4):
                w64min[a, 16 * m : 16 * m + 16, c, 4 * c + m] = 1.0
                w64abs[a, 16 * m : 16 * m + 16, c, 4 * c + m] = -0.5
    w64min = np.ascontiguousarray(w64min.reshape(P, CH * 32))
    w64abs = np.ascontiguousarray(w64abs.reshape(P, CH * 32))
    # bhalf[v]: +0.5 identity on min rows; v=0 -> r<28, v=1 -> r<24, v=2 -> all
    bh = np.zeros((P, 3, 32), np.float16)
    for v, rmin in ((0, 28), (1, 24), (2, 32)):
        for h in range(2):
            for r in range(rmin):
                bh[64 * h + r, v, r] = 0.5
    # wneg: all -1 pattern (B table build; btile = -B)
    wneg = np.zeros((2, 64, CH, 32), np.float16)
    for a in range(2):
        for c in range(CH):
            for m in range(4):
                wneg[a, 16 * m : 16 * m + 16, c, 4 * c + m] = -1.0
    wneg = np.ascontiguousarray(wneg.reshape(P, CH * 32))
    # selw[h]: scr row 32C + 4c + m  ->  k = 8c + 4h + m   (sum over C)
    selw = np.zeros((P, 2, 64), np.float32)
    for h in range(2):
        for C in range(4):
            for c in range(CH):
                for m in range(4):
                    selw[32 * C + 4 * c + m, h, 8 * c + 4 * h + m] = 1.0
    # zb: zeros row [64] + ones row [WIN] for the obJ bracket matmuls
    zb = np.concatenate(
        [np.zeros((1, 64), np.float32), np.ones((1, WIN), np.float32)], axis=1
    )
    # eraser: single contract-row weight -BIG; indicator at ebuf col 320
    bigw = np.zeros((2, 64, 4 * 32), np.float16)
    for a in range(2):
        bigw[a, 0, :] = -BIG
    bigw = np.ascontiguousarray(bigw.reshape(P, 4 * 32))
    ebuf = np.zeros((P, 608), np.float16)
    ebuf[0, 320] = 1.0
    ebuf[64, 320] = 1.0
    return w64min, w64abs, bh, wneg, selw, zb, bigw, ebuf


def _window_jmap(q):
    p4 = (q + 4) % 8
    jm = np.zeros(WIN, np.int64)
    yhalf = 32 if q < 4 else 0
    jm[0:32] = 64 * p4 + yhalf + np.arange(32)
    jm[32:96] = 64 * q + np.arange(64)
    for b in range(1, 4):
        jm[96 + 64 * (b - 1) : 96 + 64 * b] = 64 * ((q + b) % 8) + np.arange(64)
    xhalf = 0 if q < 4 else 32
    jm[288:320] = 64 * p4 + xhalf + np.arange(32)
    return jm


def kernel(x, W, b):
    import ml_dtypes

    x = np.asarray(x, np.float32)
    W = np.asarray(W, np.float32)
    if "nc" not in _cache:
        _cache["nc"] = _build()
    nc = _cache["nc"]
    w64min, w64abs, bh, wneg, selw, zb, bigw, ebuf = _consts()

    xT = np.ascontiguousarray(x.T)  # [INSIZE, N]
    wt = np.ascontiguousarray(
        W.reshape(CH, P, NT, P).transpose(0, 3, 2, 1).astype(np.float16)
    )
    jmaps = [_window_jmap(q) for q in range(NCORES)]
    in_maps = []
    for q in range(NCORES):
        xs = np.ascontiguousarray(x[NL * q : NL * (q + 1)])
        xtw = np.ascontiguousarray(
            xT[:, jmaps[q]].astype(np.float16).reshape(NT, P, WIN)
        )
        in_maps.append({
            "xtw": xtw, "wt": wt, "w64min": w64min, "w64abs": w64abs,
            "bhalf": bh, "wneg": wneg,
            "selw": selw.astype(ml_dtypes.bfloat16),
            "zb": zb.astype(ml_dtypes.bfloat16),
            "bigw": bigw, "ebuf": ebuf, "xs": xs,
        })
    res = run_bass_kernel_spmd(
        nc, in_maps, core_ids=list(range(NCORES)), trace=TRACE
    )
    _cache["last_results"] = res

    # ---------------- host assembly ----------------
    o_b = np.zeros((N, K), np.float32)
    # index maps for o_raw: row = 32C + 4c + m, col = 2p + h
    rows = np.arange(P)
    Cc = rows // 32
    cc = (rows % 32) // 4
    mm = rows % 4
    cols = np.arange(2 * NP)
    pp = cols // 2
    hh = cols % 2
    i_loc = 4 * pp[None, :] + Cc[:, None]          # [P, 32]
    k_idx = 8 * cc[:, None] + 4 * hh[None, :] + mm[:, None]  # [P, 32]
    out_rows = []
    for q in range(NCORES):
        r = res.results[q]
        oraw = np.asarray(r["oraw"], np.float32)
        obj = np.asarray(r["objo"], np.float32)
        # own-row part
        flat_i = (64 * q + i_loc).ravel()
        flat_k = k_idx.ravel()
        np.add.at(o_b, (flat_i, flat_k), oraw.ravel())
        # cross-row part
        o_b[jmaps[q]] += obj.T
        out_rows.append(np.asarray(r["outx"], np.float32))
    xs_full = np.concatenate(out_rows, axis=0)
    return np.ascontiguousarray(np.concatenate([xs_full, o_b], axis=1))
